# revision 1
# baseline (speedup 1.0000x reference)
"""BipartiteSAGEConv Trainium2 kernel.

Strategy: destination-sharded, zero collectives.
- Host: sort/partition edges by destination across 8 cores (6250 dsts each),
  group per 128-dst tile, split by src half (int16 index limit), pad to
  128-edge chunks (uniform chunk structure across cores so one SPMD program
  serves all 8 cores).
- Device per core: dma_gather (MoE row-gather ucode) pulls per-edge src rows
  HBM->SBUF; scatter-add via one-hot matmul on the TensorEngine accumulates
  [dst,128] sums + counts in PSUM; divide by count; two linear layers + bias
  via PE matmuls; DMA out the [6250,128] shard.
"""

import sys
import types

import numpy as np

N_SRC = 50000
N_DST = 50000
E = 800000
D = 128
OUT = 128
N_CORES = 8
P = 128
DST_PER_CORE = N_DST // N_CORES          # 6250
TILES = (DST_PER_CORE + P - 1) // P      # 49
HALF = 25000                             # int16 index limit split
MAX_ROWS_PER_GATHER = 1024               # SWDGE ring limit (measured)


def _install_ntff_hook():
    try:
        import antenv
        if "antenv.axon_hooks" in sys.modules:
            return
        mod = types.ModuleType("antenv.axon_hooks")
        _h = [None]
        mod.set_axon_ntff_profile_hook = lambda h: _h.__setitem__(0, h)
        mod.get_axon_ntff_profile_hook = lambda: _h[0]
        sys.modules["antenv.axon_hooks"] = mod
        antenv.axon_hooks = mod
        from trn_agent_boot.trn_boot import _ntff_profile_via_ctypes
        mod.set_axon_ntff_profile_hook(
            _ntff_profile_via_ctypes("/opt/axon/libaxon_pjrt.so"))
    except Exception:
        pass


def _prep_core(edge_src, edge_dst, core):
    """Per-core edge structure: for each (tile, half) return the edge lists.

    Returns list over 49 tiles of (src_lo, dstl_lo, src_hi, dstl_hi) where
    src_* are int64 source indices (absolute) and dstl_* are local-dst-in-tile
    ids, both sorted by dst.
    """
    lo = core * DST_PER_CORE
    m = (edge_dst >= lo) & (edge_dst < lo + DST_PER_CORE)
    es = edge_src[m]
    ed = edge_dst[m] - lo
    order = np.argsort(ed, kind="stable")
    es, ed = es[order], ed[order]
    tiles = []
    tile_id = ed >> 7
    bounds = np.searchsorted(tile_id, np.arange(TILES + 1))
    for t in range(TILES):
        a, b = bounds[t], bounds[t + 1]
        s, dl = es[a:b], ed[a:b] - t * P
        is_lo = s < HALF
        tiles.append((s[is_lo], dl[is_lo], s[~is_lo] - HALF, dl[~is_lo]))
    return tiles


def _pad_chunks(src, dstl, n_chunks):
    """Pad to n_chunks*128 edges; pad idx=0 (valid row), dstl=-1 (no one-hot)."""
    n = n_chunks * P
    s = np.zeros(n, np.int16)
    d = np.full(n, -1.0, np.float32)
    s[:len(src)] = src.astype(np.int16)
    d[:len(dstl)] = dstl.astype(np.float32)
    return s, d


def _wrap_idx(idx_flat):
    """dma_gather wrapped index layout: index j at partition j%16, col j//16,
    replicated across the 8 gpsimd cores (partition groups of 16)."""
    n = len(idx_flat)
    w = idx_flat.reshape(n // 16, 16).T          # [16, n/16]
    return np.tile(w, (8, 1))                    # [128, n/16]


def build_and_run(x_src, x_dst, edge_src, edge_dst, W_neigh, b_neigh,
                  W_self, b_self):
    _install_ntff_hook()
    from concourse import bacc, bass, mybir, tile
    from concourse.bass_utils import run_bass_kernel_spmd
    from concourse.masks import make_identity

    F32 = mybir.dt.float32

    # ---------- host-side sharding / layout ----------
    per_core_tiles = [_prep_core(edge_src, edge_dst, c) for c in range(N_CORES)]

    # uniform chunk counts across cores (SPMD: one program, 8 data sets)
    KL = [max(max(1, -(-len(per_core_tiles[c][t][0]) // P))
              for c in range(N_CORES)) for t in range(TILES)]
    KH = [max(max(1, -(-len(per_core_tiles[c][t][2]) // P))
              for c in range(N_CORES)) for t in range(TILES)]
    KE = [KL[t] + KH[t] for t in range(TILES)]
    NCH = sum(KE)                                 # total chunks per core
    KEMAX = max(KE)

    # gather plan: per tile, lo-half then hi-half, each split into gathers of
    # <= MAX_ROWS_PER_GATHER rows.  gathers: (tile, half, chunk_off_in_tile,
    # n_chunks, idx_col_base)
    gathers = []
    idx_cols = 0                                  # int16 columns consumed
    for t in range(TILES):
        off = 0
        for half, k_half in ((0, KL[t]), (1, KH[t])):
            k_done = 0
            while k_done < k_half:
                k = min(k_half - k_done, MAX_ROWS_PER_GATHER // P)
                gathers.append((t, half, off, k, idx_cols))
                idx_cols += k * 8                 # k*128/16 columns
                off += k
                k_done += k
    IDXCOLS = idx_cols

    # per-core data arrays
    idx_all = np.zeros((N_CORES, P, IDXCOLS), np.int16)
    dstl_all = np.zeros((N_CORES, P, NCH), np.float32)
    cbase = np.concatenate([[0], np.cumsum(KE)])  # chunk col base per tile
    for c in range(N_CORES):
        for t in range(TILES):
            s_lo, d_lo, s_hi, d_hi = per_core_tiles[c][t]
            sl, dl = _pad_chunks(s_lo, d_lo, KL[t])
            sh, dh = _pad_chunks(s_hi, d_hi, KH[t])
            s_cat = np.concatenate([sl, sh])
            d_cat = np.concatenate([dl, dh])
            # dstl layout: [128, NCH]; slot p of chunk k = edge k*128+p
            dstl_all[c][:, cbase[t]:cbase[t + 1]] = (
                d_cat.reshape(KE[t], P).T)
        for (t, half, off, k, colb) in gathers:
            s_lo, d_lo, s_hi, d_hi = per_core_tiles[c][t]
            sl, _ = _pad_chunks(s_lo, d_lo, KL[t])
            sh, _ = _pad_chunks(s_hi, d_hi, KH[t])
            s_cat = np.concatenate([sl, sh])
            rows = s_cat[(off) * P:(off + k) * P]
            idx_all[c][:, colb:colb + k * 8] = _wrap_idx(rows)

    x_lo = np.ascontiguousarray(x_src[:HALF]).astype(np.float32)
    x_hi = np.ascontiguousarray(x_src[HALF:]).astype(np.float32)
    xdstT = np.zeros((N_CORES, P, TILES * P), np.float32)
    for c in range(N_CORES):
        shard = x_dst[c * DST_PER_CORE:(c + 1) * DST_PER_CORE]  # [6250,128]
        xdstT[c][:, :DST_PER_CORE] = shard.T
    iota = np.tile(np.arange(P, dtype=np.float32), (P, 1))
    wn = W_neigh.astype(np.float32)
    ws = W_self.astype(np.float32)
    bsum = (b_neigh + b_self).astype(np.float32)[None, :]  # [1,128]

    # ---------- device program ----------
    nc = bacc.Bacc("TRN2", target_bir_lowering=False, debug=False,
                   num_devices=N_CORES)
    xlo_d = nc.dram_tensor("xlo", [HALF, D], F32, kind="ExternalInput").ap()
    xhi_d = nc.dram_tensor("xhi", [HALF, D], F32, kind="ExternalInput").ap()
    idx_d = nc.dram_tensor("idx", [P, IDXCOLS], mybir.dt.int16,
                           kind="ExternalInput").ap()
    dstl_d = nc.dram_tensor("dstl", [P, NCH], F32, kind="ExternalInput").ap()
    xdstT_d = nc.dram_tensor("xdstT", [P, TILES * P], F32,
                             kind="ExternalInput").ap()
    iota_d = nc.dram_tensor("iota", [P, P], F32, kind="ExternalInput").ap()
    wn_d = nc.dram_tensor("wn", [D, OUT], F32, kind="ExternalInput").ap()
    ws_d = nc.dram_tensor("ws", [D, OUT], F32, kind="ExternalInput").ap()
    bsum_d = nc.dram_tensor("bsum", [1, OUT], F32, kind="ExternalInput").ap()
    out_d = nc.dram_tensor("out", [DST_PER_CORE, OUT], F32,
                           kind="ExternalOutput").ap()

    with tile.TileContext(nc) as tc:
        with (
            tc.tile_pool(name="const", bufs=1) as cpool,
            tc.tile_pool(name="work", bufs=2) as wpool,
            tc.tile_pool(name="psum", bufs=2, space="PSUM") as ppool,
        ):
            idx_sb = cpool.tile([P, IDXCOLS], mybir.dt.int16)
            dstl_sb = cpool.tile([P, NCH], F32)
            xdstT_sb = cpool.tile([P, TILES * P], F32)
            iota_sb = cpool.tile([P, P], F32)
            wn_sb = cpool.tile([D, OUT], F32)
            ws_sb = cpool.tile([D, OUT], F32)
            bsum_sb = cpool.tile([1, OUT], F32)
            ones_sb = cpool.tile([P, 1], F32)
            ones_row = cpool.tile([1, P], F32)
            ident_sb = cpool.tile([P, P], F32)
            nc.sync.dma_start(out=idx_sb[:], in_=idx_d[:])
            nc.sync.dma_start(out=dstl_sb[:], in_=dstl_d[:])
            nc.sync.dma_start(out=xdstT_sb[:], in_=xdstT_d[:])
            nc.sync.dma_start(out=iota_sb[:], in_=iota_d[:])
            nc.sync.dma_start(out=wn_sb[:], in_=wn_d[:])
            nc.sync.dma_start(out=ws_sb[:], in_=ws_d[:])
            nc.sync.dma_start(out=bsum_sb[:], in_=bsum_d[:])
            nc.gpsimd.memset(ones_sb[:], 1.0)
            nc.gpsimd.memset(ones_row[:], 1.0)
            make_identity(nc, ident_sb[:])

            # group gathers by tile
            g_by_tile = [[] for _ in range(TILES)]
            for g in gathers:
                g_by_tile[g[0]].append(g)

            for t in range(TILES):
                ke = KE[t]
                g_sb = wpool.tile([P, KEMAX * P], F32, tag="g", name=f"g{t}")
                for (_, half, off, k, colb) in g_by_tile[t]:
                    t_ap = g_sb[:]
                    out3d = bass.AP(t_ap.tensor, t_ap.offset + off * P,
                                    [t_ap.ap[0], [P, k], [1, P]])
                    nc.gpsimd.dma_gather(
                        out3d,
                        (xlo_d if half == 0 else xhi_d)[:],
                        idx_sb[:, colb:colb + k * 8],
                        k * P,
                        k * P,
                        D,
                    )
                # batched one-hot: oh[p, k*128+j] = (iota[p,j] == dstl[p,cb+k])
                oh_sb = wpool.tile([P, KEMAX * P], F32, tag="oh", name=f"oh{t}")
                i_ap = iota_sb[:]
                iota3d = bass.AP(i_ap.tensor, i_ap.offset,
                                 [i_ap.ap[0], [0, ke], [i_ap.ap[1][0], P]])
                d_ap = dstl_sb[:]
                dstl3d = bass.AP(d_ap.tensor, d_ap.offset + int(cbase[t]),
                                 [d_ap.ap[0], [d_ap.ap[1][0], ke], [0, P]])
                oh3d = bass.AP(oh_sb[:].tensor, oh_sb[:].offset,
                               [oh_sb[:].ap[0], [P, ke], [1, P]])
                nc.vector.tensor_tensor(out=oh3d, in0=iota3d, in1=dstl3d,
                                        op=mybir.AluOpType.is_equal)

                ps1 = ppool.tile([P, 132], F32, tag="ps1", name=f"ps1_{t}",
                                 space="PSUM")
                for k in range(ke):
                    nc.tensor.matmul(
                        out=ps1[:, 0:D],
                        lhsT=oh_sb[:, k * P:(k + 1) * P],
                        rhs=g_sb[:, k * P:(k + 1) * P],
                        start=(k == 0), stop=(k == ke - 1))
                for k in range(ke):
                    nc.tensor.matmul(
                        out=ps1[:, D:D + 1],
                        lhsT=oh_sb[:, k * P:(k + 1) * P],
                        rhs=ones_sb[:],
                        start=(k == 0), stop=(k == ke - 1))

                cnt_sb = wpool.tile([P, 1], F32, tag="cnt", name=f"cnt{t}")
                nc.vector.tensor_scalar_max(out=cnt_sb[:], in0=ps1[:, D:D + 1],
                                            scalar1=1.0)
                rcnt_sb = wpool.tile([P, 1], F32, tag="rcnt", name=f"rc{t}")
                nc.vector.reciprocal(out=rcnt_sb[:], in_=cnt_sb[:])
                agg_sb = wpool.tile([P, D], F32, tag="agg", name=f"agg{t}")
                nc.vector.tensor_tensor(out=agg_sb[:], in0=ps1[:, 0:D],
                                        in1=rcnt_sb[:].to_broadcast([P, D]),
                                        op=mybir.AluOpType.mult)
                ps_t = ppool.tile([P, P], F32, tag="pst", name=f"pst{t}",
                                  space="PSUM")
                nc.tensor.transpose(out=ps_t[:], in_=agg_sb[:],
                                    identity=ident_sb[:])
                aggT_sb = wpool.tile([P, D], F32, tag="aggT", name=f"agT{t}")
                nc.vector.tensor_copy(out=aggT_sb[:], in_=ps_t[:])

                ps2 = ppool.tile([P, OUT], F32, tag="ps2", name=f"ps2_{t}",
                                 space="PSUM")
                nc.tensor.matmul(out=ps2[:], lhsT=aggT_sb[:], rhs=wn_sb[:],
                                 start=True, stop=False)
                nc.tensor.matmul(out=ps2[:],
                                 lhsT=xdstT_sb[:, t * P:(t + 1) * P],
                                 rhs=ws_sb[:], start=False, stop=False)
                nc.tensor.matmul(out=ps2[:], lhsT=ones_row[:], rhs=bsum_sb[:],
                                 start=False, stop=True)
                o_sb = wpool.tile([P, OUT], F32, tag="osb", name=f"o{t}")
                nc.scalar.copy(out=o_sb[:], in_=ps2[:])
                rows = min(P, DST_PER_CORE - t * P)
                nc.sync.dma_start(out=out_d[t * P:t * P + rows, :],
                                  in_=o_sb[:rows, :])

    nc.finalize()

    in_maps = [{
        "xlo": x_lo, "xhi": x_hi, "idx": idx_all[c], "dstl": dstl_all[c],
        "xdstT": xdstT[c], "iota": iota, "wn": wn, "ws": ws, "bsum": bsum,
    } for c in range(N_CORES)]

    import os
    trace = os.environ.get("BSAGE_TRACE", "0") == "1"
    res = run_bass_kernel_spmd(nc, in_maps, core_ids=list(range(N_CORES)),
                               trace=trace)
    out = np.concatenate([res.results[c]["out"] for c in range(N_CORES)],
                         axis=0)
    if trace:
        build_and_run.last_exec_ns = res.exec_time_ns
    return out


def kernel(x_src, x_dst, edge_src, edge_dst, num_dst, W_neigh, b_neigh,
           W_self, b_self):
    x_src = np.asarray(x_src, dtype=np.float32)
    x_dst = np.asarray(x_dst, dtype=np.float32)
    edge_src = np.asarray(edge_src).astype(np.int64)
    edge_dst = np.asarray(edge_dst).astype(np.int64)
    W_neigh = np.asarray(W_neigh, dtype=np.float32)
    b_neigh = np.asarray(b_neigh, dtype=np.float32)
    W_self = np.asarray(W_self, dtype=np.float32)
    b_self = np.asarray(b_self, dtype=np.float32)
    return build_and_run(x_src, x_dst, edge_src, edge_dst, W_neigh, b_neigh,
                         W_self, b_self)


# revision 5
# speedup vs baseline: 1.0028x; 1.0028x over previous
"""BipartiteSAGEConv Trainium2 kernel.

Strategy: destination-sharded, zero collectives.
- Host: sort/partition edges by destination across 8 cores (6250 dsts each),
  group per 128-dst tile, split by src half (int16 index limit), pad to
  128-edge chunks (uniform chunk structure across cores so one SPMD program
  serves all 8 cores).
- Device per core: dma_gather (MoE row-gather ucode) pulls per-edge src rows
  HBM->SBUF; scatter-add via one-hot matmul on the TensorEngine accumulates
  [dst,128] sums + counts in PSUM; divide by count; two linear layers + bias
  via PE matmuls; DMA out the [6250,128] shard.
"""

import sys
import types

import numpy as np

N_SRC = 50000
N_DST = 50000
E = 800000
D = 128
OUT = 128
N_CORES = 8
P = 128
DST_PER_CORE = N_DST // N_CORES          # 6250
TILES = (DST_PER_CORE + P - 1) // P      # 49
HALF = 25000                             # int16 index limit split
MAX_ROWS_PER_GATHER = 1024               # SWDGE ring limit (measured)


def _install_ntff_hook():
    try:
        import antenv
        if "antenv.axon_hooks" in sys.modules:
            return
        mod = types.ModuleType("antenv.axon_hooks")
        _h = [None]
        mod.set_axon_ntff_profile_hook = lambda h: _h.__setitem__(0, h)
        mod.get_axon_ntff_profile_hook = lambda: _h[0]
        sys.modules["antenv.axon_hooks"] = mod
        antenv.axon_hooks = mod
        from trn_agent_boot.trn_boot import _ntff_profile_via_ctypes
        mod.set_axon_ntff_profile_hook(
            _ntff_profile_via_ctypes("/opt/axon/libaxon_pjrt.so"))
    except Exception:
        pass


def _prep_core(edge_src, edge_dst, core):
    """Per-core edge structure: for each (tile, half) return the edge lists.

    Returns list over 49 tiles of (src_lo, dstl_lo, src_hi, dstl_hi) where
    src_* are int64 source indices (absolute) and dstl_* are local-dst-in-tile
    ids, both sorted by dst.
    """
    lo = core * DST_PER_CORE
    m = (edge_dst >= lo) & (edge_dst < lo + DST_PER_CORE)
    es = edge_src[m]
    ed = edge_dst[m] - lo
    order = np.argsort(ed, kind="stable")
    es, ed = es[order], ed[order]
    tiles = []
    tile_id = ed >> 7
    bounds = np.searchsorted(tile_id, np.arange(TILES + 1))
    for t in range(TILES):
        a, b = bounds[t], bounds[t + 1]
        s, dl = es[a:b], ed[a:b] - t * P
        is_lo = s < HALF
        tiles.append((s[is_lo], dl[is_lo], s[~is_lo] - HALF, dl[~is_lo]))
    return tiles


def _pad_chunks(src, dstl, n_chunks):
    """Pad to n_chunks*128 edges; pad idx=0 (valid row), dstl=-1 (no one-hot)."""
    n = n_chunks * P
    s = np.zeros(n, np.int16)
    d = np.full(n, -1.0, np.float32)
    s[:len(src)] = src.astype(np.int16)
    d[:len(dstl)] = dstl.astype(np.float32)
    return s, d


def _wrap_idx(idx_flat):
    """dma_gather wrapped index layout: index j at partition j%16, col j//16,
    replicated across the 8 gpsimd cores (partition groups of 16)."""
    n = len(idx_flat)
    w = idx_flat.reshape(n // 16, 16).T          # [16, n/16]
    return np.tile(w, (8, 1))                    # [128, n/16]


def build_and_run(x_src, x_dst, edge_src, edge_dst, W_neigh, b_neigh,
                  W_self, b_self):
    _install_ntff_hook()
    from concourse import bacc, bass, mybir, tile
    from concourse.bass_utils import run_bass_kernel_spmd
    from concourse.masks import make_identity

    F32 = mybir.dt.float32

    # ---------- host-side sharding / layout ----------
    per_core_tiles = [_prep_core(edge_src, edge_dst, c) for c in range(N_CORES)]

    # uniform chunk counts across cores (SPMD: one program, 8 data sets)
    KL = [max(max(1, -(-len(per_core_tiles[c][t][0]) // P))
              for c in range(N_CORES)) for t in range(TILES)]
    KH = [max(max(1, -(-len(per_core_tiles[c][t][2]) // P))
              for c in range(N_CORES)) for t in range(TILES)]
    KE = [KL[t] + KH[t] for t in range(TILES)]
    NCH = sum(KE)                                 # total chunks per core
    KEMAX = max(KE)

    # exact row counts per (tile, half): max over cores, rounded to 16
    NLO = [max(len(per_core_tiles[c][t][0]) for c in range(N_CORES))
           for t in range(TILES)]
    NHI = [max(len(per_core_tiles[c][t][2]) for c in range(N_CORES))
           for t in range(TILES)]
    NLO = [min(KL[t] * P, max(16, -(-n // 16) * 16)) for t, n in enumerate(NLO)]
    NHI = [min(KH[t] * P, max(16, -(-n // 16) * 16)) for t, n in enumerate(NHI)]

    # gather plan: per tile, lo-half then hi-half, each split into gathers of
    # <= MAX_ROWS_PER_GATHER rows.  gathers: (tile, half, chunk_off_in_tile,
    # n_chunks, idx_col_base, num_idxs)
    gathers = []
    idx_cols = 0                                  # int16 columns consumed
    for t in range(TILES):
        off = 0
        for half, k_half, n_half in ((0, KL[t], NLO[t]), (1, KH[t], NHI[t])):
            k_done = 0
            while k_done * P < n_half:
                nrows = min(n_half - k_done * P, MAX_ROWS_PER_GATHER)
                k = -(-nrows // P)
                gathers.append((t, half, off, k, idx_cols, nrows))
                idx_cols += -(-nrows // 16)
                off += k
                k_done += k
            off = k_half if half == 0 else off    # hi starts at chunk KL[t]
    IDXCOLS = idx_cols

    # per-core data arrays
    idx_all = np.zeros((N_CORES, P, IDXCOLS), np.int16)
    dstl_all = np.zeros((N_CORES, P, NCH), np.float32)
    cbase = np.concatenate([[0], np.cumsum(KE)])  # chunk col base per tile
    for c in range(N_CORES):
        for t in range(TILES):
            s_lo, d_lo, s_hi, d_hi = per_core_tiles[c][t]
            sl, dl = _pad_chunks(s_lo, d_lo, KL[t])
            sh, dh = _pad_chunks(s_hi, d_hi, KH[t])
            s_cat = np.concatenate([sl, sh])
            d_cat = np.concatenate([dl, dh])
            # dstl layout: [128, NCH]; slot p of chunk k = edge k*128+p
            dstl_all[c][:, cbase[t]:cbase[t + 1]] = (
                d_cat.reshape(KE[t], P).T)
        for (t, half, off, k, colb, nrows) in gathers:
            s_lo, d_lo, s_hi, d_hi = per_core_tiles[c][t]
            sl, _ = _pad_chunks(s_lo, d_lo, KL[t])
            sh, _ = _pad_chunks(s_hi, d_hi, KH[t])
            base = off * P if half == 0 else (off - KL[t]) * P
            src_half = sl if half == 0 else sh
            rows = src_half[base:base + nrows]
            ncols = -(-nrows // 16)
            idx_all[c][:, colb:colb + ncols] = _wrap_idx(rows)

    x_lo = np.ascontiguousarray(x_src[:HALF]).astype(np.float32)
    x_hi = np.ascontiguousarray(x_src[HALF:]).astype(np.float32)
    xdstT = np.zeros((N_CORES, P, TILES * P), np.float32)
    for c in range(N_CORES):
        shard = x_dst[c * DST_PER_CORE:(c + 1) * DST_PER_CORE]  # [6250,128]
        xdstT[c][:, :DST_PER_CORE] = shard.T
    iota = np.tile(np.arange(P, dtype=np.float32), (P, 1))
    wn = W_neigh.astype(np.float32)
    ws = W_self.astype(np.float32)
    bsum = (b_neigh + b_self).astype(np.float32)[None, :]  # [1,128]

    # ---------- device program ----------
    nc = bacc.Bacc("TRN2", target_bir_lowering=False, debug=False,
                   num_devices=N_CORES)
    xlo_d = nc.dram_tensor("xlo", [HALF, D], F32, kind="ExternalInput").ap()
    xhi_d = nc.dram_tensor("xhi", [HALF, D], F32, kind="ExternalInput").ap()
    idx_d = nc.dram_tensor("idx", [P, IDXCOLS], mybir.dt.int16,
                           kind="ExternalInput").ap()
    dstl_d = nc.dram_tensor("dstl", [P, NCH], F32, kind="ExternalInput").ap()
    xdstT_d = nc.dram_tensor("xdstT", [P, TILES * P], F32,
                             kind="ExternalInput").ap()
    iota_d = nc.dram_tensor("iota", [P, P], F32, kind="ExternalInput").ap()
    wn_d = nc.dram_tensor("wn", [D, OUT], F32, kind="ExternalInput").ap()
    ws_d = nc.dram_tensor("ws", [D, OUT], F32, kind="ExternalInput").ap()
    bsum_d = nc.dram_tensor("bsum", [1, OUT], F32, kind="ExternalInput").ap()
    out_d = nc.dram_tensor("out", [DST_PER_CORE, OUT], F32,
                           kind="ExternalOutput").ap()

    with tile.TileContext(nc) as tc:
        with (
            tc.tile_pool(name="const", bufs=1) as cpool,
            tc.tile_pool(name="work", bufs=2) as wpool,
            tc.tile_pool(name="psum", bufs=2, space="PSUM") as ppool,
        ):
            idx_sb = cpool.tile([P, IDXCOLS], mybir.dt.int16)
            dstl_sb = cpool.tile([P, NCH], F32)
            xdstT_sb = cpool.tile([P, TILES * P], F32)
            iota_sb = cpool.tile([P, P], F32)
            wn_sb = cpool.tile([D, OUT], F32)
            ws_sb = cpool.tile([D, OUT], F32)
            bsum_sb = cpool.tile([1, OUT], F32)
            ones_sb = cpool.tile([P, 1], F32)
            ones_row = cpool.tile([1, P], F32)
            ident_sb = cpool.tile([P, P], F32)
            nc.sync.dma_start(out=idx_sb[:], in_=idx_d[:])
            nc.sync.dma_start(out=dstl_sb[:], in_=dstl_d[:])
            nc.sync.dma_start(out=xdstT_sb[:], in_=xdstT_d[:])
            nc.sync.dma_start(out=iota_sb[:], in_=iota_d[:])
            nc.sync.dma_start(out=wn_sb[:], in_=wn_d[:])
            nc.sync.dma_start(out=ws_sb[:], in_=ws_d[:])
            nc.sync.dma_start(out=bsum_sb[:], in_=bsum_d[:])
            nc.vector.memset(ones_sb[:], 1.0)
            nc.vector.memset(ones_row[:], 1.0)
            make_identity(nc, ident_sb[:])

            # group gathers by tile
            g_by_tile = [[] for _ in range(TILES)]
            for g in gathers:
                g_by_tile[g[0]].append(g)

            for t in range(TILES):
                ke = KE[t]
                g_sb = wpool.tile([P, KEMAX * P], F32, tag="g", name=f"g{t}")
                if t < 2:
                    # first use of each rotating slot: clear so that slots the
                    # gather never writes hold finite values (0 * x = 0)
                    nc.vector.memset(g_sb[:], 0.0)
                for (_, half, off, k, colb, nrows) in g_by_tile[t]:
                    t_ap = g_sb[:]
                    out3d = bass.AP(t_ap.tensor, t_ap.offset + off * P,
                                    [t_ap.ap[0], [P, k], [1, P]])
                    nc.gpsimd.dma_gather(
                        out3d,
                        (xlo_d if half == 0 else xhi_d)[:],
                        idx_sb[:, colb:colb + (-(-nrows // 16))],
                        nrows,
                        nrows,
                        D,
                    )
                # batched one-hot: oh[p, k*128+j] = (iota[p,j] == dstl[p,cb+k])
                oh_sb = wpool.tile([P, KEMAX * P], F32, tag="oh", name=f"oh{t}")
                i_ap = iota_sb[:]
                iota3d = bass.AP(i_ap.tensor, i_ap.offset,
                                 [i_ap.ap[0], [0, ke], [i_ap.ap[1][0], P]])
                d_ap = dstl_sb[:]
                dstl3d = bass.AP(d_ap.tensor, d_ap.offset + int(cbase[t]),
                                 [d_ap.ap[0], [d_ap.ap[1][0], ke], [0, P]])
                oh3d = bass.AP(oh_sb[:].tensor, oh_sb[:].offset,
                               [oh_sb[:].ap[0], [P, ke], [1, P]])
                nc.vector.tensor_tensor(out=oh3d, in0=iota3d, in1=dstl3d,
                                        op=mybir.AluOpType.is_equal)

                ps1 = ppool.tile([P, 132], F32, tag="ps1", name=f"ps1_{t}",
                                 space="PSUM")
                for k in range(ke):
                    nc.tensor.matmul(
                        out=ps1[:, 0:D],
                        lhsT=oh_sb[:, k * P:(k + 1) * P],
                        rhs=g_sb[:, k * P:(k + 1) * P],
                        start=(k == 0), stop=(k == ke - 1))
                for k in range(ke):
                    nc.tensor.matmul(
                        out=ps1[:, D:D + 1],
                        lhsT=oh_sb[:, k * P:(k + 1) * P],
                        rhs=ones_sb[:],
                        start=(k == 0), stop=(k == ke - 1))

                cnt_sb = wpool.tile([P, 1], F32, tag="cnt", name=f"cnt{t}")
                nc.vector.tensor_scalar_max(out=cnt_sb[:], in0=ps1[:, D:D + 1],
                                            scalar1=1.0)
                rcnt_sb = wpool.tile([P, 1], F32, tag="rcnt", name=f"rc{t}")
                nc.vector.reciprocal(out=rcnt_sb[:], in_=cnt_sb[:])
                agg_sb = wpool.tile([P, D], F32, tag="agg", name=f"agg{t}")
                nc.vector.tensor_tensor(out=agg_sb[:], in0=ps1[:, 0:D],
                                        in1=rcnt_sb[:].to_broadcast([P, D]),
                                        op=mybir.AluOpType.mult)
                ps_t = ppool.tile([P, P], F32, tag="pst", name=f"pst{t}",
                                  space="PSUM")
                nc.tensor.transpose(out=ps_t[:], in_=agg_sb[:],
                                    identity=ident_sb[:])
                aggT_sb = wpool.tile([P, D], F32, tag="aggT", name=f"agT{t}")
                nc.vector.tensor_copy(out=aggT_sb[:], in_=ps_t[:])

                ps2 = ppool.tile([P, OUT], F32, tag="ps2", name=f"ps2_{t}",
                                 space="PSUM")
                nc.tensor.matmul(out=ps2[:], lhsT=aggT_sb[:], rhs=wn_sb[:],
                                 start=True, stop=False)
                nc.tensor.matmul(out=ps2[:],
                                 lhsT=xdstT_sb[:, t * P:(t + 1) * P],
                                 rhs=ws_sb[:], start=False, stop=False)
                nc.tensor.matmul(out=ps2[:], lhsT=ones_row[:], rhs=bsum_sb[:],
                                 start=False, stop=True)
                o_sb = wpool.tile([P, OUT], F32, tag="osb", name=f"o{t}")
                nc.scalar.copy(out=o_sb[:], in_=ps2[:])
                rows = min(P, DST_PER_CORE - t * P)
                nc.sync.dma_start(out=out_d[t * P:t * P + rows, :],
                                  in_=o_sb[:rows, :])

    nc.finalize()

    in_maps = [{
        "xlo": x_lo, "xhi": x_hi, "idx": idx_all[c], "dstl": dstl_all[c],
        "xdstT": xdstT[c], "iota": iota, "wn": wn, "ws": ws, "bsum": bsum,
    } for c in range(N_CORES)]

    import os
    trace = os.environ.get("BSAGE_TRACE", "0") == "1"
    res = run_bass_kernel_spmd(nc, in_maps, core_ids=list(range(N_CORES)),
                               trace=trace)
    out = np.concatenate([res.results[c]["out"] for c in range(N_CORES)],
                         axis=0)
    if trace:
        build_and_run.last_exec_ns = res.exec_time_ns
    return out


def kernel(x_src, x_dst, edge_src, edge_dst, num_dst, W_neigh, b_neigh,
           W_self, b_self):
    x_src = np.asarray(x_src, dtype=np.float32)
    x_dst = np.asarray(x_dst, dtype=np.float32)
    edge_src = np.asarray(edge_src).astype(np.int64)
    edge_dst = np.asarray(edge_dst).astype(np.int64)
    W_neigh = np.asarray(W_neigh, dtype=np.float32)
    b_neigh = np.asarray(b_neigh, dtype=np.float32)
    W_self = np.asarray(W_self, dtype=np.float32)
    b_self = np.asarray(b_self, dtype=np.float32)
    return build_and_run(x_src, x_dst, edge_src, edge_dst, W_neigh, b_neigh,
                         W_self, b_self)


# revision 6
# speedup vs baseline: 2.0955x; 2.0898x over previous
"""BipartiteSAGEConv Trainium2 kernel.

Strategy: destination-sharded, zero collectives.
- Host: sort/partition edges by destination across 8 cores (6250 dsts each),
  group per 128-dst tile, split by src half (int16 index limit), pad to
  128-edge chunks (uniform chunk structure across cores so one SPMD program
  serves all 8 cores).
- Device per core: dma_gather (MoE row-gather ucode) pulls per-edge src rows
  HBM->SBUF; scatter-add via one-hot matmul on the TensorEngine accumulates
  [dst,128] sums + counts in PSUM; divide by count; two linear layers + bias
  via PE matmuls; DMA out the [6250,128] shard.
"""

import sys
import types

import numpy as np

N_SRC = 50000
N_DST = 50000
E = 800000
D = 128
OUT = 128
N_CORES = 8
P = 128
DST_PER_CORE = N_DST // N_CORES          # 6250
TILES = (DST_PER_CORE + P - 1) // P      # 49
HALF = 25000                             # int16 index limit split
MAX_ROWS_PER_GATHER = 1024               # SWDGE ring limit (measured)


def _install_ntff_hook():
    try:
        import antenv
        if "antenv.axon_hooks" in sys.modules:
            return
        mod = types.ModuleType("antenv.axon_hooks")
        _h = [None]
        mod.set_axon_ntff_profile_hook = lambda h: _h.__setitem__(0, h)
        mod.get_axon_ntff_profile_hook = lambda: _h[0]
        sys.modules["antenv.axon_hooks"] = mod
        antenv.axon_hooks = mod
        from trn_agent_boot.trn_boot import _ntff_profile_via_ctypes
        mod.set_axon_ntff_profile_hook(
            _ntff_profile_via_ctypes("/opt/axon/libaxon_pjrt.so"))
    except Exception:
        pass


def _prep_core(edge_src, edge_dst, core):
    """Per-core edge structure: for each (tile, half) return the edge lists.

    Returns list over 49 tiles of (src_lo, dstl_lo, src_hi, dstl_hi) where
    src_* are int64 source indices (absolute) and dstl_* are local-dst-in-tile
    ids, both sorted by dst.
    """
    lo = core * DST_PER_CORE
    m = (edge_dst >= lo) & (edge_dst < lo + DST_PER_CORE)
    es = edge_src[m]
    ed = edge_dst[m] - lo
    order = np.argsort(ed, kind="stable")
    es, ed = es[order], ed[order]
    tiles = []
    tile_id = ed >> 7
    bounds = np.searchsorted(tile_id, np.arange(TILES + 1))
    for t in range(TILES):
        a, b = bounds[t], bounds[t + 1]
        s, dl = es[a:b], ed[a:b] - t * P
        is_lo = s < HALF
        tiles.append((s[is_lo], dl[is_lo], s[~is_lo] - HALF, dl[~is_lo]))
    return tiles


def _pad_chunks(src, dstl, n_chunks):
    """Pad to n_chunks*128 edges; pad idx=0 (valid row), dstl=-1 (no one-hot)."""
    n = n_chunks * P
    s = np.zeros(n, np.int16)
    d = np.full(n, -1.0, np.float32)
    s[:len(src)] = src.astype(np.int16)
    d[:len(dstl)] = dstl.astype(np.float32)
    return s, d


def _wrap_idx(idx_flat):
    """dma_gather wrapped index layout: index j at partition j%16, col j//16,
    replicated across the 8 gpsimd cores (partition groups of 16)."""
    n = len(idx_flat)
    w = idx_flat.reshape(n // 16, 16).T          # [16, n/16]
    return np.tile(w, (8, 1))                    # [128, n/16]


def build_and_run(x_src, x_dst, edge_src, edge_dst, W_neigh, b_neigh,
                  W_self, b_self):
    _install_ntff_hook()
    from concourse import bacc, bass, mybir, tile
    from concourse.bass_utils import run_bass_kernel_spmd
    from concourse.masks import make_identity

    F32 = mybir.dt.float32
    import os as _os
    use_f16 = _os.environ.get("BSAGE_F32", "0") != "1"
    DTAB = mybir.dt.float16 if use_f16 else F32
    np_tab = np.float16 if use_f16 else np.float32

    # ---------- host-side sharding / layout ----------
    per_core_tiles = [_prep_core(edge_src, edge_dst, c) for c in range(N_CORES)]

    # uniform chunk counts across cores (SPMD: one program, 8 data sets)
    KL = [max(max(1, -(-len(per_core_tiles[c][t][0]) // P))
              for c in range(N_CORES)) for t in range(TILES)]
    KH = [max(max(1, -(-len(per_core_tiles[c][t][2]) // P))
              for c in range(N_CORES)) for t in range(TILES)]
    KE = [KL[t] + KH[t] for t in range(TILES)]
    NCH = sum(KE)                                 # total chunks per core
    KEMAX = max(KE)

    # exact row counts per (tile, half): max over cores, rounded to 16
    NLO = [max(len(per_core_tiles[c][t][0]) for c in range(N_CORES))
           for t in range(TILES)]
    NHI = [max(len(per_core_tiles[c][t][2]) for c in range(N_CORES))
           for t in range(TILES)]
    NLO = [min(KL[t] * P, max(16, -(-n // 16) * 16)) for t, n in enumerate(NLO)]
    NHI = [min(KH[t] * P, max(16, -(-n // 16) * 16)) for t, n in enumerate(NHI)]

    # gather plan: per tile, lo-half then hi-half, each split into gathers of
    # <= MAX_ROWS_PER_GATHER rows.  gathers: (tile, half, chunk_off_in_tile,
    # n_chunks, idx_col_base, num_idxs)
    gathers = []
    idx_cols = 0                                  # int16 columns consumed
    for t in range(TILES):
        off = 0
        for half, k_half, n_half in ((0, KL[t], NLO[t]), (1, KH[t], NHI[t])):
            k_done = 0
            while k_done * P < n_half:
                nrows = min(n_half - k_done * P, MAX_ROWS_PER_GATHER)
                k = -(-nrows // P)
                gathers.append((t, half, off, k, idx_cols, nrows))
                idx_cols += -(-nrows // 16)
                off += k
                k_done += k
            off = k_half if half == 0 else off    # hi starts at chunk KL[t]
    IDXCOLS = idx_cols

    # per-core data arrays
    idx_all = np.zeros((N_CORES, P, IDXCOLS), np.int16)
    dstl_all = np.zeros((N_CORES, P, NCH), np.float32)
    cbase = np.concatenate([[0], np.cumsum(KE)])  # chunk col base per tile
    for c in range(N_CORES):
        for t in range(TILES):
            s_lo, d_lo, s_hi, d_hi = per_core_tiles[c][t]
            sl, dl = _pad_chunks(s_lo, d_lo, KL[t])
            sh, dh = _pad_chunks(s_hi, d_hi, KH[t])
            s_cat = np.concatenate([sl, sh])
            d_cat = np.concatenate([dl, dh])
            # dstl layout: [128, NCH]; slot p of chunk k = edge k*128+p
            dstl_all[c][:, cbase[t]:cbase[t + 1]] = (
                d_cat.reshape(KE[t], P).T)
        for (t, half, off, k, colb, nrows) in gathers:
            s_lo, d_lo, s_hi, d_hi = per_core_tiles[c][t]
            sl, _ = _pad_chunks(s_lo, d_lo, KL[t])
            sh, _ = _pad_chunks(s_hi, d_hi, KH[t])
            base = off * P if half == 0 else (off - KL[t]) * P
            src_half = sl if half == 0 else sh
            rows = src_half[base:base + nrows]
            ncols = -(-nrows // 16)
            idx_all[c][:, colb:colb + ncols] = _wrap_idx(rows)

    x_lo = np.ascontiguousarray(x_src[:HALF]).astype(np_tab)
    x_hi = np.ascontiguousarray(x_src[HALF:]).astype(np_tab)
    xdstT = np.zeros((N_CORES, P, TILES * P), np.float32)
    for c in range(N_CORES):
        shard = x_dst[c * DST_PER_CORE:(c + 1) * DST_PER_CORE]  # [6250,128]
        xdstT[c][:, :DST_PER_CORE] = shard.T
    iota = np.tile(np.arange(P, dtype=np.float32), (P, 1))
    wn = W_neigh.astype(np.float32)
    ws = W_self.astype(np.float32)
    bsum = (b_neigh + b_self).astype(np.float32)[None, :]  # [1,128]

    # ---------- device program ----------
    nc = bacc.Bacc("TRN2", target_bir_lowering=False, debug=False,
                   num_devices=N_CORES, num_swdge_queues=4)
    xlo_d = nc.dram_tensor("xlo", [HALF, D], DTAB, kind="ExternalInput").ap()
    xhi_d = nc.dram_tensor("xhi", [HALF, D], DTAB, kind="ExternalInput").ap()
    idx_d = nc.dram_tensor("idx", [P, IDXCOLS], mybir.dt.int16,
                           kind="ExternalInput").ap()
    dstl_d = nc.dram_tensor("dstl", [P, NCH], F32, kind="ExternalInput").ap()
    xdstT_d = nc.dram_tensor("xdstT", [P, TILES * P], F32,
                             kind="ExternalInput").ap()
    iota_d = nc.dram_tensor("iota", [P, P], F32, kind="ExternalInput").ap()
    wn_d = nc.dram_tensor("wn", [D, OUT], F32, kind="ExternalInput").ap()
    ws_d = nc.dram_tensor("ws", [D, OUT], F32, kind="ExternalInput").ap()
    bsum_d = nc.dram_tensor("bsum", [1, OUT], F32, kind="ExternalInput").ap()
    out_d = nc.dram_tensor("out", [DST_PER_CORE, OUT], F32,
                           kind="ExternalOutput").ap()

    with tile.TileContext(nc) as tc:
        with (
            tc.tile_pool(name="const", bufs=1) as cpool,
            tc.tile_pool(name="work", bufs=2) as wpool,
            tc.tile_pool(name="psum", bufs=2, space="PSUM") as ppool,
        ):
            idx_sb = cpool.tile([P, IDXCOLS], mybir.dt.int16)
            dstl_sb = cpool.tile([P, NCH], F32)
            xdstT_sb = cpool.tile([P, TILES * P], F32)
            iota_sb = cpool.tile([P, P], F32)
            wn_sb = cpool.tile([D, OUT], F32)
            ws_sb = cpool.tile([D, OUT], F32)
            bsum_sb = cpool.tile([1, OUT], F32)
            ones_sb = cpool.tile([P, 1], DTAB)
            ones_row = cpool.tile([1, P], F32)
            ident_sb = cpool.tile([P, P], F32)
            nc.sync.dma_start(out=idx_sb[:], in_=idx_d[:])
            nc.sync.dma_start(out=dstl_sb[:], in_=dstl_d[:])
            nc.sync.dma_start(out=xdstT_sb[:], in_=xdstT_d[:])
            nc.sync.dma_start(out=iota_sb[:], in_=iota_d[:])
            nc.sync.dma_start(out=wn_sb[:], in_=wn_d[:])
            nc.sync.dma_start(out=ws_sb[:], in_=ws_d[:])
            nc.sync.dma_start(out=bsum_sb[:], in_=bsum_d[:])
            nc.vector.memset(ones_sb[:], 1.0)
            nc.vector.memset(ones_row[:], 1.0)
            make_identity(nc, ident_sb[:])

            gq = [0]
            # group gathers by tile
            g_by_tile = [[] for _ in range(TILES)]
            for g in gathers:
                g_by_tile[g[0]].append(g)

            for t in range(TILES):
                ke = KE[t]
                g_sb = wpool.tile([P, KEMAX * P], DTAB, tag="g", name=f"g{t}")
                if t < 2:
                    # first use of each rotating slot: clear so that slots the
                    # gather never writes hold finite values (0 * x = 0)
                    nc.vector.memset(g_sb[:], 0.0)
                for (_, half, off, k, colb, nrows) in g_by_tile[t]:
                    t_ap = g_sb[:]
                    out3d = bass.AP(t_ap.tensor, t_ap.offset + off * P,
                                    [t_ap.ap[0], [P, k], [1, P]])
                    nc.gpsimd.dma_gather(
                        out3d,
                        (xlo_d if half == 0 else xhi_d)[:],
                        idx_sb[:, colb:colb + (-(-nrows // 16))],
                        nrows,
                        nrows,
                        D,
                        queue_num=(gq[0] % 4),
                    )
                    gq[0] += 1
                # batched one-hot: oh[p, k*128+j] = (iota[p,j] == dstl[p,cb+k])
                oh_sb = wpool.tile([P, KEMAX * P], DTAB, tag="oh", name=f"oh{t}")
                i_ap = iota_sb[:]
                iota3d = bass.AP(i_ap.tensor, i_ap.offset,
                                 [i_ap.ap[0], [0, ke], [i_ap.ap[1][0], P]])
                d_ap = dstl_sb[:]
                dstl3d = bass.AP(d_ap.tensor, d_ap.offset + int(cbase[t]),
                                 [d_ap.ap[0], [d_ap.ap[1][0], ke], [0, P]])
                oh3d = bass.AP(oh_sb[:].tensor, oh_sb[:].offset,
                               [oh_sb[:].ap[0], [P, ke], [1, P]])
                nc.vector.tensor_tensor(out=oh3d, in0=iota3d, in1=dstl3d,
                                        op=mybir.AluOpType.is_equal)

                ps1 = ppool.tile([P, 132], F32, tag="ps1", name=f"ps1_{t}",
                                 space="PSUM")
                for k in range(ke):
                    nc.tensor.matmul(
                        out=ps1[:, 0:D],
                        lhsT=oh_sb[:, k * P:(k + 1) * P],
                        rhs=g_sb[:, k * P:(k + 1) * P],
                        start=(k == 0), stop=(k == ke - 1))
                for k in range(ke):
                    nc.tensor.matmul(
                        out=ps1[:, D:D + 1],
                        lhsT=oh_sb[:, k * P:(k + 1) * P],
                        rhs=ones_sb[:],
                        start=(k == 0), stop=(k == ke - 1))

                cnt_sb = wpool.tile([P, 1], F32, tag="cnt", name=f"cnt{t}")
                nc.vector.tensor_scalar_max(out=cnt_sb[:], in0=ps1[:, D:D + 1],
                                            scalar1=1.0)
                rcnt_sb = wpool.tile([P, 1], F32, tag="rcnt", name=f"rc{t}")
                nc.vector.reciprocal(out=rcnt_sb[:], in_=cnt_sb[:])
                agg_sb = wpool.tile([P, D], F32, tag="agg", name=f"agg{t}")
                nc.vector.tensor_tensor(out=agg_sb[:], in0=ps1[:, 0:D],
                                        in1=rcnt_sb[:].to_broadcast([P, D]),
                                        op=mybir.AluOpType.mult)
                ps_t = ppool.tile([P, P], F32, tag="pst", name=f"pst{t}",
                                  space="PSUM")
                nc.tensor.transpose(out=ps_t[:], in_=agg_sb[:],
                                    identity=ident_sb[:])
                aggT_sb = wpool.tile([P, D], F32, tag="aggT", name=f"agT{t}")
                nc.vector.tensor_copy(out=aggT_sb[:], in_=ps_t[:])

                ps2 = ppool.tile([P, OUT], F32, tag="ps2", name=f"ps2_{t}",
                                 space="PSUM")
                nc.tensor.matmul(out=ps2[:], lhsT=aggT_sb[:], rhs=wn_sb[:],
                                 start=True, stop=False)
                nc.tensor.matmul(out=ps2[:],
                                 lhsT=xdstT_sb[:, t * P:(t + 1) * P],
                                 rhs=ws_sb[:], start=False, stop=False)
                nc.tensor.matmul(out=ps2[:], lhsT=ones_row[:], rhs=bsum_sb[:],
                                 start=False, stop=True)
                o_sb = wpool.tile([P, OUT], F32, tag="osb", name=f"o{t}")
                nc.scalar.copy(out=o_sb[:], in_=ps2[:])
                rows = min(P, DST_PER_CORE - t * P)
                nc.sync.dma_start(out=out_d[t * P:t * P + rows, :],
                                  in_=o_sb[:rows, :])

    nc.finalize()

    in_maps = [{
        "xlo": x_lo, "xhi": x_hi, "idx": idx_all[c], "dstl": dstl_all[c],
        "xdstT": xdstT[c], "iota": iota, "wn": wn, "ws": ws, "bsum": bsum,
    } for c in range(N_CORES)]

    import os
    trace = os.environ.get("BSAGE_TRACE", "0") == "1"
    res = run_bass_kernel_spmd(nc, in_maps, core_ids=list(range(N_CORES)),
                               trace=trace)
    out = np.concatenate([res.results[c]["out"] for c in range(N_CORES)],
                         axis=0)
    if trace:
        build_and_run.last_exec_ns = res.exec_time_ns
    return out


def kernel(x_src, x_dst, edge_src, edge_dst, num_dst, W_neigh, b_neigh,
           W_self, b_self):
    x_src = np.asarray(x_src, dtype=np.float32)
    x_dst = np.asarray(x_dst, dtype=np.float32)
    edge_src = np.asarray(edge_src).astype(np.int64)
    edge_dst = np.asarray(edge_dst).astype(np.int64)
    W_neigh = np.asarray(W_neigh, dtype=np.float32)
    b_neigh = np.asarray(b_neigh, dtype=np.float32)
    W_self = np.asarray(W_self, dtype=np.float32)
    b_self = np.asarray(b_self, dtype=np.float32)
    return build_and_run(x_src, x_dst, edge_src, edge_dst, W_neigh, b_neigh,
                         W_self, b_self)
